# revision 11
# baseline (speedup 1.0000x reference)
"""Approximate EMD loss (entropic Sinkhorn, 50 iters) on 8 TRN2 NeuronCores.

Pure data parallel: batch b -> core b. Each core runs a 2048x2048 Sinkhorn
entirely out of SBUF:
  - K = exp(-cost/eps) stored bf16 in BOTH orientations (K^T for the row
    update, K for the column update) so every matvec runs the TensorE fast
    path: vector stationary [128,1], matrix moving [128,512] (1 col/cycle).
  - The log-domain updates collapse to multiplicative form:
        e^u = C / (K e^v + eps_log),  C = 1/N + eps_log
    done by a fused ScalarE Reciprocal (PSUM -> SBUF row), then PE
    transposes reshape the [1,512] row chunks into [128,1] stationary
    columns for the next matvec.
  - Final EMD = e^u^T (K*cost) e^v with K*cost recomputed blockwise from
    K via cost = -eps*ln(max(K, tiny)) (exact 0 where K underflowed).
"""

import numpy as np

N = 2048
PB = 128                  # partition block
CHW = 512                 # psum chunk width (fp32 bank limit)
ITERS = 50
EPS_SINKHORN = 0.01
EPS_LOG = 1e-8
NCORES = 8


def _host_prep(X1, X2, n):
    """Per-batch host-side input prep (cheap O(N))."""
    X1 = np.ascontiguousarray(X1, dtype=np.float32)
    X2 = np.ascontiguousarray(X2, dtype=np.float32)
    A = (X1 * X1).sum(1).astype(np.float32)   # |x1_i|^2
    Bv = (X2 * X2).sum(1).astype(np.float32)  # |x2_j|^2
    ones = np.ones((1, n), np.float32)
    nb = n // PB
    # Layout A (K[i,j], i on partitions):  P' = x1e . x2e  with
    #   x1e=[x1,1], x2e=[x2,-B/2]  =>  K = exp(200*P' - 100*A_i)
    L1 = np.concatenate([X1.T, ones], 0)                  # [4, n] stationary
    R1 = np.concatenate([X2.T, (-Bv / 2)[None, :]], 0)    # [4, n] moving
    # Layout B (K^T[j,i], j on partitions): symmetric roles
    L2 = np.concatenate([X2.T, ones], 0)
    R2 = np.concatenate([X1.T, (-A / 2)[None, :]], 0)
    scl = np.float32(-2.0 / EPS_SINKHORN)  # -200; bias = scl*A/2... see below
    biasA = (-A / EPS_SINKHORN).astype(np.float32).reshape(nb, PB).T.copy()
    biasB = (-Bv / EPS_SINKHORN).astype(np.float32).reshape(nb, PB).T.copy()
    return {
        "L1": L1, "R1": R1, "L2": L2, "R2": R2,
        "biasA": np.ascontiguousarray(biasA),
        "biasB": np.ascontiguousarray(biasB),
    }


def build(nc, tc, ctx, aps, n=N, iters=ITERS):
    """Emit the single-core program. aps: dict name->dram AP."""
    import concourse.mybir as mybir

    f32 = mybir.dt.float32
    bf16 = mybir.dt.bfloat16
    AF = mybir.ActivationFunctionType
    ALU = mybir.AluOpType

    nb = n // PB            # number of 128-blocks
    nch = n // CHW          # number of 512-chunks
    tpc = CHW // PB         # transposes per chunk (4)
    C_MU = float(1.0 / n + EPS_LOG)
    ESCL = float(2.0 / EPS_SINKHORN)    # 200.0

    persist = ctx.enter_context(tc.tile_pool(name="persist", bufs=1))

    KA = persist.tile([PB, nb * n], bf16, tag="KA")   # [i_p, ib*n + j]
    KB = persist.tile([PB, nb * n], bf16, tag="KB")   # [j_p, jb*n + i]
    ev = persist.tile([PB, nb], bf16, tag="ev")       # e^v stationary cols
    eu = persist.tile([PB, nb], bf16, tag="eu")       # e^u stationary cols
    ident = persist.tile([1, 1], f32, tag="ident")
    ones_col = persist.tile([PB, 1], f32, tag="ones_col")
    biasA_sb = persist.tile([PB, nb], f32, tag="biasA")
    biasB_sb = persist.tile([PB, nb], f32, tag="biasB")
    eu32 = persist.tile([PB, nb], f32, tag="eu32")
    persist_ps = ctx.enter_context(
        tc.tile_pool(name="persist_ps", bufs=1, space="PSUM"))
    wcol = persist_ps.tile([PB, nb], f32, tag="wcol")

    nc.gpsimd.memset(ident[:, :], 1.0)
    nc.gpsimd.memset(ones_col[:, :], 1.0)
    nc.gpsimd.memset(ev[:, :], 1.0)   # e^{v_0} = 1
    nc.sync.dma_start(out=biasA_sb[:, :], in_=aps["biasA"][:, :])
    nc.sync.dma_start(out=biasB_sb[:, :], in_=aps["biasB"][:, :])

    # ---------------- setup: K in both layouts ----------------
    with tc.tile_pool(name="sin", bufs=1) as sin, \
         tc.tile_pool(name="spsum", bufs=3, space="PSUM") as sp:
        L1 = sin.tile([4, n], f32, tag="L1")
        R1 = sin.tile([4, n], f32, tag="R1")
        L2 = sin.tile([4, n], f32, tag="L2")
        R2 = sin.tile([4, n], f32, tag="R2")
        for t, name in ((L1, "L1"), (R1, "R1"), (L2, "L2"), (R2, "R2")):
            nc.sync.dma_start(out=t[:, :], in_=aps[name][:, :])
        for L, R, bias, KT in ((L1, R1, biasA_sb, KA), (L2, R2, biasB_sb, KB)):
            for ib in range(nb):
                for jc in range(nch):
                    P = sp.tile([PB, CHW], f32, tag="P")
                    nc.tensor.matmul(
                        P[:, :],
                        lhsT=L[:, ib * PB:(ib + 1) * PB],
                        rhs=R[:, jc * CHW:(jc + 1) * CHW],
                        start=True, stop=True,
                    )
                    nc.scalar.activation(
                        KT[:, ib * n + jc * CHW: ib * n + (jc + 1) * CHW],
                        P[:, :], AF.Exp,
                        bias=bias[:, ib:ib + 1], scale=ESCL,
                    )

    # ---------------- Sinkhorn iterations ----------------
    rp = ctx.enter_context(tc.tile_pool(name="rp", bufs=5, space="PSUM"))
    tp = ctx.enter_context(tc.tile_pool(name="tp", bufs=2, space="PSUM"))
    rows = ctx.enter_context(tc.tile_pool(name="rows", bufs=4))

    def half(mat, src, dst):
        """dst[:, :] (bf16 cols) = C / (matvec(mat, src) + eps)."""
        pending = None
        for c in range(nch):
            r = rp.tile([1, CHW], f32, tag="r")
            for jb in range(nb):
                nc.tensor.matmul(
                    r[0:1, :],
                    lhsT=src[:, jb:jb + 1],
                    rhs=mat[:, jb * n + c * CHW: jb * n + (c + 1) * CHW],
                    start=(jb == 0), stop=(jb == nb - 1),
                )
            if pending is not None:
                pending()
            def transform(c=c, r=r):
                # row = (r + eps)/C  (fused into the PSUM->SBUF copy)
                row = rows.tile([1, CHW], f32, tag="brow")
                nc.scalar.activation(
                    row[0:1, :], r[0:1, :], AF.Copy,
                    bias=EPS_LOG / C_MU, scale=1.0 / C_MU,
                )
                tcol = tp.tile([PB, tpc], f32, tag="tcol")
                for t in range(tpc):
                    nc.tensor.transpose(
                        tcol[:, t:t + 1],
                        row[0:1, t * PB:(t + 1) * PB],
                        ident[0:1, 0:1],
                    )
                rec = rows.tile([PB, tpc], f32, tag="rec")
                nc.vector.reciprocal(rec[:, :], tcol[:, :])
                nc.vector.tensor_copy(dst[:, c * tpc:(c + 1) * tpc], rec[:, :])
            pending = transform
        pending()

    for _ in range(iters):
        half(KB, ev, eu)   # u-update: r_i = sum_j K[i,j] e^{v_j}
        half(KA, eu, ev)   # v-update: c_j = sum_i K[i,j] e^{u_i}

    # ---------------- final: emd = e^u^T (K*cost) e^v ----------------
    with tc.tile_pool(name="fin", bufs=2) as fin:
        nc.vector.tensor_copy(eu32[:, :], eu[:, :])
        ws = []
        for c in range(nch):
            ws.append(rp.tile([1, CHW], f32, tag="r", name=f"w{c}"))
        for jb in range(nb):
            kb_blk = KB[:, jb * n:(jb + 1) * n]
            kcl = fin.tile([PB, n], bf16, tag="kcl")
            nc.vector.tensor_scalar_max(kcl[:, :], kb_blk, 2e-38)
            lnk = fin.tile([PB, n], f32, tag="lnk")
            nc.scalar.activation(lnk[:, :], kcl[:, :], AF.Ln)
            t2 = fin.tile([PB, n], f32, tag="t2")
            nc.vector.tensor_scalar_mul(t2[:, :], lnk[:, :], -EPS_SINKHORN)
            mt = fin.tile([PB, n], bf16, tag="mt")   # (K*cost)^T block
            nc.vector.tensor_mul(mt[:, :], kb_blk, t2[:, :])
            for c in range(nch):
                nc.tensor.matmul(
                    ws[c][0:1, :],
                    lhsT=ev[:, jb:jb + 1],
                    rhs=mt[:, c * CHW:(c + 1) * CHW],
                    start=(jb == 0), stop=(jb == nb - 1),
                )
        for c in range(nch):
            row = rows.tile([1, CHW], f32, tag="brow")
            nc.scalar.activation(row[0:1, :], ws[c][0:1, :], AF.Copy)
            for t in range(tpc):
                nc.tensor.transpose(
                    wcol[:, c * tpc + t: c * tpc + t + 1],
                    row[0:1, t * PB:(t + 1) * PB],
                    ident[0:1, 0:1],
                )
        prod = fin.tile([PB, nb], f32, tag="prod")
        dots = fin.tile([PB, 1], f32, tag="dots")
        nc.vector.tensor_mul(prod[:, :], wcol[:, :], eu32[:, :])
        nc.vector.reduce_sum(dots[:, :], prod[:, :], axis=mybir.AxisListType.X)
        emd_ps = tp.tile([1, 1], f32, tag="tcol", name="emd_ps")
        nc.tensor.matmul(emd_ps[0:1, 0:1], lhsT=dots[:, 0:1],
                         rhs=ones_col[:, 0:1], start=True, stop=True)
        out_sb = fin.tile([1, 1], f32, tag="out_sb")
        nc.scalar.activation(out_sb[0:1, :], emd_ps[0:1, :], AF.Copy)
        nc.sync.dma_start(out=aps["out"][:, :], in_=out_sb[0:1, :])


def _build_program(n=N, iters=ITERS, debug=False):
    from contextlib import ExitStack
    import concourse.mybir as mybir
    import concourse.tile as tile
    from concourse import bacc

    f32 = mybir.dt.float32
    nb = n // PB
    nc = bacc.Bacc(
        "TRN2",
        target_bir_lowering=False,
        debug=debug,
        enable_asserts=True,
        num_devices=NCORES,
    )
    aps = {}
    for name in ("L1", "R1", "L2", "R2"):
        aps[name] = nc.dram_tensor(name, [4, n], f32, kind="ExternalInput")[:, :]
    for name in ("biasA", "biasB"):
        aps[name] = nc.dram_tensor(name, [PB, nb], f32, kind="ExternalInput")[:, :]
    aps["out"] = nc.dram_tensor("out", [1, 1], f32, kind="ExternalOutput")[:, :]
    with ExitStack() as ctx:
        tc = ctx.enter_context(tile.TileContext(nc))
        build(nc, tc, ctx, aps, n=n, iters=iters)
    nc.compile()
    return nc


_CACHE = {}
LAST_RESULT = None


def kernel(x1, x2):
    global LAST_RESULT
    from concourse.bass_utils import run_bass_kernel_spmd

    x1 = np.asarray(x1, dtype=np.float32)
    x2 = np.asarray(x2, dtype=np.float32)
    B = x1.shape[0]
    assert B == NCORES and x1.shape[1] == N

    if "nc" not in _CACHE:
        _CACHE["nc"] = _build_program()
    nc = _CACHE["nc"]

    in_maps = [_host_prep(x1[b], x2[b], N) for b in range(B)]
    res = run_bass_kernel_spmd(nc, in_maps, core_ids=list(range(NCORES)))
    LAST_RESULT = res
    out = np.array([res.results[b]["out"][0, 0] for b in range(B)],
                   dtype=np.float32)
    return out


if __name__ == "__main__":
    rng = np.random.default_rng(0)
    x1 = rng.standard_normal((NCORES, N, 3)).astype(np.float32)
    x2 = rng.standard_normal((NCORES, N, 3)).astype(np.float32)
    print(kernel(x1, x2))


# revision 15
# speedup vs baseline: 1.0298x; 1.0298x over previous
"""Approximate EMD loss (entropic Sinkhorn, 50 iters) on 8 TRN2 NeuronCores.

Pure data parallel: batch b -> core b. Each core runs a 2048x2048 Sinkhorn
entirely out of SBUF:
  - K = exp(-cost/eps) stored bf16 in BOTH orientations (K^T for the row
    update, K for the column update) so every matvec runs the TensorE fast
    path: vector stationary [128,1], matrix moving [128,512] (1 col/cycle).
  - The log-domain updates collapse to multiplicative form:
        e^u = C / (K e^v + eps_log),  C = 1/N + eps_log
    done by a fused ScalarE Reciprocal (PSUM -> SBUF row), then PE
    transposes reshape the [1,512] row chunks into [128,1] stationary
    columns for the next matvec.
  - Final EMD = e^u^T (K*cost) e^v with K*cost recomputed blockwise from
    K via cost = -eps*ln(max(K, tiny)) (exact 0 where K underflowed).
"""

import numpy as np

N = 2048
PB = 128                  # partition block
CHW = 512                 # psum chunk width (fp32 bank limit)
ITERS = 50
EPS_SINKHORN = 0.01
EPS_LOG = 1e-8
NCORES = 8


def _host_prep(X1, X2, n):
    """Per-batch host-side input prep (cheap O(N))."""
    X1 = np.ascontiguousarray(X1, dtype=np.float32)
    X2 = np.ascontiguousarray(X2, dtype=np.float32)
    A = (X1 * X1).sum(1).astype(np.float32)   # |x1_i|^2
    Bv = (X2 * X2).sum(1).astype(np.float32)  # |x2_j|^2
    ones = np.ones((1, n), np.float32)
    nb = n // PB
    # Layout A (K[i,j], i on partitions):  P' = x1e . x2e  with
    #   x1e=[x1,1], x2e=[x2,-B/2]  =>  K = exp(200*P' - 100*A_i)
    L1 = np.concatenate([X1.T, ones], 0)                  # [4, n] stationary
    R1 = np.concatenate([X2.T, (-Bv / 2)[None, :]], 0)    # [4, n] moving
    biasA = (-A / EPS_SINKHORN).astype(np.float32).reshape(nb, PB).T.copy()
    return {
        "L1": L1, "R1": R1,
        "biasA": np.ascontiguousarray(biasA),
    }


def build(nc, tc, ctx, aps, n=N, iters=ITERS):
    """Emit the single-core program. aps: dict name->dram AP."""
    import concourse.mybir as mybir

    f32 = mybir.dt.float32
    bf16 = mybir.dt.bfloat16
    AF = mybir.ActivationFunctionType
    ALU = mybir.AluOpType

    nb = n // PB            # number of 128-blocks
    nch = n // CHW          # number of 512-chunks
    tpc = CHW // PB         # transposes per chunk (4)
    C_MU = float(1.0 / n + EPS_LOG)
    ESCL = float(2.0 / EPS_SINKHORN)    # 200.0

    persist = ctx.enter_context(tc.tile_pool(name="persist", bufs=1))

    KA = persist.tile([PB, nb * n], bf16, tag="KA")   # [i_p, ib*n + j]
    KB = persist.tile([PB, nb * n], bf16, tag="KB")   # [j_p, jb*n + i]
    ev = persist.tile([PB, nb], bf16, tag="ev")       # e^v stationary cols
    eu = persist.tile([PB, nb], bf16, tag="eu")       # e^u stationary cols
    ident = persist.tile([1, 1], f32, tag="ident")
    identB = persist.tile([PB, PB], bf16, tag="identB")
    ones_col = persist.tile([PB, 1], f32, tag="ones_col")
    biasA_sb = persist.tile([PB, nb], f32, tag="biasA")
    eu32 = persist.tile([PB, nb], f32, tag="eu32")
    persist_ps = ctx.enter_context(
        tc.tile_pool(name="persist_ps", bufs=1, space="PSUM"))
    wcol = persist_ps.tile([PB, nb], f32, tag="wcol")

    from concourse.masks import make_identity

    nc.gpsimd.memset(ident[:, :], 1.0)
    nc.gpsimd.memset(ones_col[:, :], 1.0)
    nc.gpsimd.memset(ev[:, :], 1.0)   # e^{v_0} = 1
    make_identity(nc, identB[:, :])
    nc.sync.dma_start(out=biasA_sb[:, :], in_=aps["biasA"][:, :])

    # ---------------- setup: K_A via matmul+exp; K_B by transposing ----------
    with tc.tile_pool(name="sin", bufs=1) as sin, \
         tc.tile_pool(name="spsum", bufs=3, space="PSUM") as sp:
        L1 = sin.tile([4, n], f32, tag="L1")
        R1 = sin.tile([4, n], f32, tag="R1")
        for t, name in ((L1, "L1"), (R1, "R1")):
            nc.sync.dma_start(out=t[:, :], in_=aps[name][:, :])
        pending = None
        for ib in range(nb):
            for jc in range(nch):
                P = sp.tile([PB, CHW], f32, tag="P")
                nc.tensor.matmul(
                    P[:, :],
                    lhsT=L1[:, ib * PB:(ib + 1) * PB],
                    rhs=R1[:, jc * CHW:(jc + 1) * CHW],
                    start=True, stop=True,
                )
                nc.scalar.activation(
                    KA[:, ib * n + jc * CHW: ib * n + (jc + 1) * CHW],
                    P[:, :], AF.Exp,
                    bias=biasA_sb[:, ib:ib + 1], scale=ESCL,
                )
                if pending is not None:
                    pending()
                def mk_transpose(ib=ib, jc=jc):
                    # K_B[j, i] tiles by transposing the just-built K_A chunk
                    for q in range(tpc):
                        kbt = sp.tile([PB, PB], bf16, tag="kbt", name="kbt")
                        nc.tensor.transpose(
                            kbt[:, :],
                            KA[:, ib * n + jc * CHW + q * PB:
                               ib * n + jc * CHW + (q + 1) * PB],
                            identB[:, :],
                        )
                        nc.vector.tensor_copy(
                            KB[:, (jc * tpc + q) * n + ib * PB:
                               (jc * tpc + q) * n + (ib + 1) * PB],
                            kbt[:, :],
                        )
                pending = mk_transpose
        pending()

    # ---------------- Sinkhorn iterations ----------------
    rp = ctx.enter_context(tc.tile_pool(name="rp", bufs=5, space="PSUM"))
    tp = ctx.enter_context(tc.tile_pool(name="tp", bufs=2, space="PSUM"))
    rows = ctx.enter_context(tc.tile_pool(name="rows", bufs=4))

    def half(mat, src, dst):
        """dst[:, :] (bf16 cols) = C / (matvec(mat, src) + eps)."""
        pending = None
        for c in range(nch):
            r = rp.tile([1, CHW], f32, tag="r")
            for jb in range(nb):
                nc.tensor.matmul(
                    r[0:1, :],
                    lhsT=src[:, jb:jb + 1],
                    rhs=mat[:, jb * n + c * CHW: jb * n + (c + 1) * CHW],
                    start=(jb == 0), stop=(jb == nb - 1),
                )
            if pending is not None:
                pending()
            def transform(c=c, r=r):
                # row = (r + eps)/C  (fused into the PSUM->SBUF copy)
                row = rows.tile([1, CHW], f32, tag="brow")
                nc.scalar.activation(
                    row[0:1, :], r[0:1, :], AF.Copy,
                    bias=EPS_LOG / C_MU, scale=1.0 / C_MU,
                )
                tcol = tp.tile([PB, tpc], f32, tag="tcol")
                for t in range(tpc):
                    nc.tensor.transpose(
                        tcol[:, t:t + 1],
                        row[0:1, t * PB:(t + 1) * PB],
                        ident[0:1, 0:1],
                    )
                rec = rows.tile([PB, tpc], f32, tag="rec")
                nc.vector.reciprocal(rec[:, :], tcol[:, :])
                nc.vector.tensor_copy(dst[:, c * tpc:(c + 1) * tpc], rec[:, :])
            pending = transform
        pending()

    for _ in range(iters):
        half(KB, ev, eu)   # u-update: r_i = sum_j K[i,j] e^{v_j}
        half(KA, eu, ev)   # v-update: c_j = sum_i K[i,j] e^{u_i}

    # ---------------- final: emd = e^u^T (K*cost) e^v ----------------
    with tc.tile_pool(name="fin", bufs=2) as fin:
        nc.vector.tensor_copy(eu32[:, :], eu[:, :])
        ws = []
        for c in range(nch):
            ws.append(rp.tile([1, CHW], f32, tag="r", name=f"w{c}"))
        for jb in range(nb):
            kb_blk = KB[:, jb * n:(jb + 1) * n]
            kcl = fin.tile([PB, n], bf16, tag="kcl")
            nc.vector.tensor_scalar_max(kcl[:, :], kb_blk, 2e-38)
            lnk = fin.tile([PB, n], f32, tag="lnk")
            nc.scalar.activation(lnk[:, :], kcl[:, :], AF.Ln)
            t2 = fin.tile([PB, n], f32, tag="t2")
            nc.vector.tensor_scalar_mul(t2[:, :], lnk[:, :], -EPS_SINKHORN)
            mt = fin.tile([PB, n], bf16, tag="mt")   # (K*cost)^T block
            nc.vector.tensor_mul(mt[:, :], kb_blk, t2[:, :])
            for c in range(nch):
                nc.tensor.matmul(
                    ws[c][0:1, :],
                    lhsT=ev[:, jb:jb + 1],
                    rhs=mt[:, c * CHW:(c + 1) * CHW],
                    start=(jb == 0), stop=(jb == nb - 1),
                )
        for c in range(nch):
            row = rows.tile([1, CHW], f32, tag="brow")
            nc.scalar.activation(row[0:1, :], ws[c][0:1, :], AF.Copy)
            for t in range(tpc):
                nc.tensor.transpose(
                    wcol[:, c * tpc + t: c * tpc + t + 1],
                    row[0:1, t * PB:(t + 1) * PB],
                    ident[0:1, 0:1],
                )
        prod = fin.tile([PB, nb], f32, tag="prod")
        dots = fin.tile([PB, 1], f32, tag="dots")
        nc.vector.tensor_mul(prod[:, :], wcol[:, :], eu32[:, :])
        nc.vector.reduce_sum(dots[:, :], prod[:, :], axis=mybir.AxisListType.X)
        emd_ps = tp.tile([1, 1], f32, tag="tcol", name="emd_ps")
        nc.tensor.matmul(emd_ps[0:1, 0:1], lhsT=dots[:, 0:1],
                         rhs=ones_col[:, 0:1], start=True, stop=True)
        out_sb = fin.tile([1, 1], f32, tag="out_sb")
        nc.scalar.activation(out_sb[0:1, :], emd_ps[0:1, :], AF.Copy)
        nc.sync.dma_start(out=aps["out"][:, :], in_=out_sb[0:1, :])


def _build_program(n=N, iters=ITERS, debug=False):
    from contextlib import ExitStack
    import concourse.mybir as mybir
    import concourse.tile as tile
    from concourse import bacc

    f32 = mybir.dt.float32
    nb = n // PB
    nc = bacc.Bacc(
        "TRN2",
        target_bir_lowering=False,
        debug=debug,
        enable_asserts=True,
        num_devices=NCORES,
    )
    aps = {}
    for name in ("L1", "R1"):
        aps[name] = nc.dram_tensor(name, [4, n], f32, kind="ExternalInput")[:, :]
    for name in ("biasA",):
        aps[name] = nc.dram_tensor(name, [PB, nb], f32, kind="ExternalInput")[:, :]
    aps["out"] = nc.dram_tensor("out", [1, 1], f32, kind="ExternalOutput")[:, :]
    with ExitStack() as ctx:
        tc = ctx.enter_context(tile.TileContext(nc))
        build(nc, tc, ctx, aps, n=n, iters=iters)
    nc.compile()
    return nc


_CACHE = {}
LAST_RESULT = None


def kernel(x1, x2):
    global LAST_RESULT
    from concourse.bass_utils import run_bass_kernel_spmd

    x1 = np.asarray(x1, dtype=np.float32)
    x2 = np.asarray(x2, dtype=np.float32)
    B = x1.shape[0]
    assert B == NCORES and x1.shape[1] == N

    if "nc" not in _CACHE:
        _CACHE["nc"] = _build_program()
    nc = _CACHE["nc"]

    in_maps = [_host_prep(x1[b], x2[b], N) for b in range(B)]
    res = run_bass_kernel_spmd(nc, in_maps, core_ids=list(range(NCORES)))
    LAST_RESULT = res
    out = np.array([res.results[b]["out"][0, 0] for b in range(B)],
                   dtype=np.float32)
    return out


if __name__ == "__main__":
    rng = np.random.default_rng(0)
    x1 = rng.standard_normal((NCORES, N, 3)).astype(np.float32)
    x2 = rng.standard_normal((NCORES, N, 3)).astype(np.float32)
    print(kernel(x1, x2))
